# revision 16
# baseline (speedup 1.0000x reference)
"""Two-layer GAT on 8 Trainium2 NeuronCores (Bass/Tile).

Strategy (per core, nodes dst-sharded 8 ways):
 - Span-aligned schedule: tiles of <=128 edges whose dsts lie in one aligned
   32-dst span of a 128-dst window. All PSUM column offsets are compile-time
   (no PE register loads).
 - Source rows fetched with dma_gather (512B rows [x bf16 | a_src f32]) on
   4 SWDGE queues round-robin with deep xg buffering: the SDMA drain of the
   random-row descriptors is the bottleneck (~3-4 ns/row), so keep all four
   descriptor rings backed up and everything else overlapped beneath.
 - Aggregation as matmuls against host-built 0/1 span indicators S
   ([128 edges x 32 dst] bf16, resident); a_dst broadcast to edges via
   ST32^T matmuls (ST32 = per-tile [32, 128] transposed indicator, packed
   4 tiles per 128-partition block, streamed from DRAM).
 - Self-loop edges never enter the gather: their contribution is one
   diagonal-matrix matmul per window (which doubles as the PSUM init).
 - Layer 1 aggregates raw bf16 features (linearity trick), applies W1 in the
   window epilogue. Layer 2 aggregates rank-12 projected rows
   [h1@W2 | a_src2 | 1]: the ones column yields the softmax denominator.
"""
import math
import os
import numpy as np
import ml_dtypes

import concourse.bass as bass
import concourse.bacc as bacc
import concourse.tile as tile
from concourse import mybir
from concourse.bass_utils import run_bass_kernel_spmd

F32 = mybir.dt.float32
BF16 = mybir.dt.bfloat16
I16 = mybir.dt.int16
I32 = mybir.dt.int32

NQ = 4           # SWDGE queues for gather round-robin
CALL_CAP = 4     # tiles per gather call (512 idxs: ring capacity)
XG_BUFS = 12     # gather landing strips in flight


class Cfg:
    def __init__(self, N, E, IN, HID, HEADS, NCLS, n_cores=8, win=128,
                 ws=32, neg_slope=0.2):
        assert IN == 128, "kernel assumes 128 input features"
        self.N, self.E, self.IN, self.HID, self.HEADS, self.NCLS = N, E, IN, HID, HEADS, NCLS
        self.n_cores = n_cores
        self.shard = N // n_cores
        self.win = win                      # dsts per PSUM window
        self.ws = ws                        # dsts per tile span
        self.nw = math.ceil(self.shard / win)
        self.nspan = win // ws
        self.half_split = 25000             # int16 gather base split
        self.neg_slope = neg_slope
        self.tb = 12                        # tiles per score batch


def _wrap_idx(idx128):
    """128 int16 idxs -> [16, 8] wrapped, tiled to [128, 8]."""
    w = idx128.reshape(8, 16).T
    return np.tile(w, (8, 1)).astype(np.int16)


def preprocess(cfg, edge_index):
    """Span-aligned schedule, uniform across cores.

    sched: list of (w, span, half, T) with T = tiles (max over cores).
    calls: list of (slot0_tile, ntiles, half) gather calls.
    aux[c]: idx [128, 8*TT] i16, S [128, ws*TT] bf16,
            ST32 [128, 128*ceil(TT/4)] bf16 (tile t in partition block t%4).
    """
    N, ncores, shard, win, ws = cfg.N, cfg.n_cores, cfg.shard, cfg.win, cfg.ws
    HS = cfg.half_split
    SRC = np.asarray(edge_index[0], np.int64)
    DST = np.asarray(edge_index[1], np.int64)
    nw32 = math.ceil(shard / ws)            # 32-spans per core

    # per-core edge lists grouped by (span32, half)
    counts = np.zeros((ncores, nw32, 2), np.int64)
    per_core = []
    for c in range(ncores):
        m = (DST // shard) == c
        s = SRC[m]
        dl = DST[m] - c * shard
        sp = dl // ws
        half = (s >= HS).astype(np.int64)
        order = np.lexsort((s, dl, half, sp))
        s, dl, sp, half = s[order], dl[order], sp[order], half[order]
        idx16 = np.where(half == 0, s, s - HS).astype(np.int16)
        key = sp * 2 + half
        bounds = np.searchsorted(key, np.arange(nw32 * 2 + 1))
        np.add.at(counts, (np.full(len(s), c), sp, half), 1)
        per_core.append((idx16, dl, bounds))

    Tgrid = np.ceil(counts.max(axis=0) / 128).astype(np.int64)  # [nw32, 2]
    # schedule in (window, half, span) order so same-half tiles are contiguous
    sched = []
    for w in range(cfg.nw):
        for f in range(2):
            for k in range(cfg.nspan):
                sp = w * cfg.nspan + k
                if sp >= nw32:
                    continue
                T = int(Tgrid[sp, f])
                if T > 0:
                    sched.append((w, k, f, T))
    TT = sum(T for _, _, _, T in sched)

    # gather calls: contiguous runs of same-half tiles, <= CALL_CAP tiles
    calls = []
    t0 = 0
    i = 0
    while i < len(sched):
        f = sched[i][2]
        run_T = 0
        j = i
        while j < len(sched) and sched[j][2] == f and run_T + sched[j][3] <= CALL_CAP:
            run_T += sched[j][3]
            j += 1
        assert run_T > 0, f"group T={sched[i][3]} exceeds CALL_CAP"
        calls.append((t0, run_T, f))
        t0 += run_T
        i = j

    nstb = math.ceil(TT / 4)
    aux = []
    for c in range(ncores):
        idx16, dl, bounds = per_core[c]
        idx_a = np.zeros((128, 8 * TT), np.int16)
        S_a = np.zeros((128, cfg.ws * TT), ml_dtypes.bfloat16)
        ST_a = np.zeros((128, 128 * TT), ml_dtypes.bfloat16)
        gt = 0
        for (w, k, f, T) in sched:
            sp = w * cfg.nspan + k
            lo, hi = bounds[sp * 2 + f], bounds[sp * 2 + f + 1]
            for t in range(T):
                a = lo + t * 128
                b = min(lo + (t + 1) * 128, hi)
                n = max(0, b - a)
                x_t = np.zeros(128, np.int16)
                if n > 0:
                    x_t[:n] = idx16[a:b]
                    d_loc = dl[a:b] - sp * ws          # in [0, ws)
                    d_win = dl[a:b] - w * win          # in [0, win)
                    e = np.arange(n)
                    S_a[e, cfg.ws * gt + d_loc] = 1
                    ST_a[d_win, 128 * gt + e] = 1
                idx_a[:, 8 * gt:8 * gt + 8] = _wrap_idx(x_t)
                gt += 1
        aux.append(dict(idx=idx_a, S=S_a, ST=ST_a))
    return sched, calls, TT, aux


def _ek4():
    E = np.zeros((128, 512), ml_dtypes.bfloat16)
    for k in range(4):
        for j in range(32):
            for b in range(4):
                E[32 * k + j, 128 * k + 32 * b + j] = 1
    return E


def fold_params(cfg, W1, as1, ad1, W2, as2, ad2):
    H, C = cfg.HEADS, cfg.HID
    Vs1 = np.stack([W1[:, h * C:(h + 1) * C] @ as1[h] for h in range(H)], 1)
    Vd1 = np.stack([W1[:, h * C:(h + 1) * C] @ ad1[h] for h in range(H)], 1)
    Vs2 = (W2 @ as2[0])[:, None]
    Vd2 = (W2 @ ad2[0])[:, None]
    V1 = np.concatenate([Vs1, Vd1], 1).astype(np.float32)        # [128, 8]
    V2W = np.concatenate([W2, Vs2, Vd2], 1).astype(np.float32)   # [128, 12]
    return V1, V2W


def build_program(cfg, sched, calls, TT):
    import contextlib
    nc = bacc.Bacc("TRN2", target_bir_lowering=False, debug=False,
                   enable_asserts=True, num_devices=cfg.n_cores,
                   dynamic_dma_scratch_size=32768, num_swdge_queues=NQ)
    N, shard, win, ws, H, tb = cfg.N, cfg.shard, cfg.win, cfg.ws, cfg.HEADS, cfg.tb
    nw, NCLS, HS = cfg.nw, cfg.NCLS, cfg.half_split
    AW = H * win      # layer-1 agg psum width (512)
    L2C = NCLS + 2
    nstb = math.ceil(TT / 4)

    xsb = nc.dram_tensor("xsb", [N, 128], F32, kind="ExternalInput").ap()
    x_loc = nc.dram_tensor("x_loc", [shard, 128], F32, kind="ExternalInput").ap()
    idx_d = nc.dram_tensor("idx", [128, 8 * TT], I16, kind="ExternalInput").ap()
    S_d = nc.dram_tensor("S", [128, ws * TT], BF16, kind="ExternalInput").ap()
    ST_d = nc.dram_tensor("ST", [128, 128 * TT], BF16, kind="ExternalInput").ap()
    Vsd1 = nc.dram_tensor("Vsd1", [128, 2 * H], BF16, kind="ExternalInput").ap()
    V2W_d = nc.dram_tensor("V2W", [128, L2C], BF16, kind="ExternalInput").ap()
    I128b_d = nc.dram_tensor("I128b", [128, 128], BF16, kind="ExternalInput").ap()
    W1b_d = nc.dram_tensor("W1b", [128, H * cfg.HID], BF16, kind="ExternalInput").ap()
    B1_d = nc.dram_tensor("B1r", [H, H * cfg.HID], F32, kind="ExternalInput").ap()
    b2r_d = nc.dram_tensor("b2rep", [128, NCLS], F32, kind="ExternalInput").ap()
    I128_d = nc.dram_tensor("I128", [128, 128], F32, kind="ExternalInput").ap()
    onesb_d = nc.dram_tensor("onesb", [128, 128], BF16, kind="ExternalInput").ap()
    Ek_d = nc.dram_tensor("Ek4", [128, 512], BF16, kind="ExternalInput").ap()
    zeros_d = nc.dram_tensor("zerosb", [128, AW], BF16, kind="ExternalInput").ap()
    out_d = nc.dram_tensor("out", [shard, NCLS], F32, kind="ExternalOutput").ap()

    with tile.TileContext(nc) as tc, contextlib.ExitStack() as ctx:
        res = ctx.enter_context(tc.tile_pool(name="res", bufs=1))
        xgp = ctx.enter_context(tc.tile_pool(name="xgp", bufs=XG_BUFS))
        stream = ctx.enter_context(tc.tile_pool(name="stream", bufs=XG_BUFS))
        work = ctx.enter_context(tc.tile_pool(name="work", bufs=2))
        psA = ctx.enter_context(tc.tile_pool(name="psA", bufs=2, space="PSUM"))
        psB = ctx.enter_context(tc.tile_pool(name="psB", bufs=1, space="PSUM"))
        dram = ctx.enter_context(tc.tile_pool(name="dram", bufs=1, space="DRAM"))

        def ld(name, shape, dt, src):
            t = res.tile(shape, dt, tag=name, name=name)
            nc.sync.dma_start(out=t[:, :], in_=src[:, :])
            return t

        idx_sb = ld("idx", [128, 8 * TT], I16, idx_d)
        S_sb = ld("S", [128, ws * TT], BF16, S_d)
        V1_sb = ld("V1", [128, 2 * H], BF16, Vsd1)
        V2W_sb = ld("V2W", [128, L2C], BF16, V2W_d)
        I128b = ld("I128b", [128, 128], BF16, I128b_d)
        W1b = ld("W1b", [128, H * cfg.HID], BF16, W1b_d)
        B1r = ld("B1r", [H, H * cfg.HID], F32, B1_d)
        b2rep = ld("b2rep", [128, NCLS], F32, b2r_d)
        I128 = ld("I128", [128, 128], F32, I128_d)
        onesb = ld("onesb", [128, 128], BF16, onesb_d)
        zerosb = ld("zerosb", [128, AW], BF16, zeros_d)
        Ek4 = ld("Ek4", [128, 512], BF16, Ek_d)

        # resident per-node tables (zero-init: pad rows must stay harmless)
        as1_blk = res.tile([128, 4 * nw], F32, tag="as1")
        nc.vector.memset(as1_blk[:, :], 0.0)
        ad1_all = res.tile([128, 4 * nw], BF16, tag="ad1")
        nc.vector.memset(ad1_all[:, :], 0.0)
        ad1f = res.tile([128, 4 * nw], F32, tag="ad1f")
        nc.vector.memset(ad1f[:, :], 0.0)
        ad2_all = res.tile([128, nw], BF16, tag="ad2")
        nc.vector.memset(ad2_all[:, :], 0.0)
        as2_self = res.tile([128, nw], F32, tag="as2s")
        nc.vector.memset(as2_self[:, :], 0.0)
        ad2f = res.tile([128, nw], F32, tag="ad2f")
        nc.vector.memset(ad2f[:, :], 0.0)
        xlb_all = res.tile([128, 128 * nw], BF16, tag="xlb")
        nc.vector.memset(xlb_all[:, :], 0.0)
        rowb_all = res.tile([128, L2C * nw], BF16, tag="rowb")
        nc.vector.memset(rowb_all[:, :], 0.0)

        sc_sh = dram.tile([shard, 4], F32)
        sc_full = dram.tile([N, 4], F32, addr_space="Shared")
        h1_sh = dram.tile([shard, 64], F32)
        h1_full = dram.tile([N, 64], F32, addr_space="Shared")

        def nrows_of(w):
            return min(win, shard - w * win)

        # ---------- Phase B: local-node layer-1 scores ----------
        for w in range(nw):
            nr = nrows_of(w)
            xl = work.tile([128, 128], F32, tag="xl")
            nc.sync.dma_start(out=xl[:nr, :], in_=x_loc[w * win:w * win + nr, :])
            nc.vector.tensor_copy(xlb_all[:nr, 128 * w:128 * w + 128], xl[:nr, :])
            xT_ps = psB.tile([128, 128], BF16, tag="scrb")
            nc.tensor.transpose(xT_ps[:, :nr], xlb_all[:nr, 128 * w:128 * w + 128],
                                I128b[:nr, :nr])
            xT = work.tile([128, 128], BF16, tag="xT")
            nc.vector.tensor_copy(xT[:, :nr], xT_ps[:, :nr])
            sc_ps = psB.tile([128, 2 * H], F32, tag="scr")
            nc.tensor.matmul(sc_ps[:nr, :], xT[:, :nr], V1_sb[:, :],
                             start=True, stop=True)
            nc.vector.tensor_copy(as1_blk[:nr, 4 * w:4 * w + 4], sc_ps[:nr, 0:4])
            nc.vector.tensor_copy(ad1_all[:nr, 4 * w:4 * w + 4], sc_ps[:nr, 4:8])
            nc.vector.tensor_copy(ad1f[:nr, 4 * w:4 * w + 4], sc_ps[:nr, 4:8])
            nc.sync.dma_start(out=sc_sh[w * win:w * win + nr, :],
                              in_=as1_blk[:nr, 4 * w:4 * w + 4])

        nc.gpsimd.collective_compute(
            "AllGather", mybir.AluOpType.bypass,
            replica_groups=[list(range(cfg.n_cores))],
            ins=[sc_sh.opt()], outs=[sc_full.opt()])

        # scatter a_src1 into xsb[:, 64:68] (f32 rows at 512B stride)
        nfull = N // 128
        ntail = N - nfull * 128
        scf = res.tile([128, 4 * (nfull + 1)], F32, tag="scf")
        nc.sync.dma_start(
            out=scf[:, 0:4 * nfull].rearrange("p (c h) -> p c h", h=4),
            in_=sc_full[0:nfull * 128, :].rearrange("(c p) h -> p c h", p=128))
        nc.sync.dma_start(
            out=xsb[0:nfull * 128, 64:68].rearrange("(c p) h -> p c h", p=128),
            in_=scf[:, 0:4 * nfull].rearrange("p (c h) -> p c h", h=4))
        if ntail:
            nc.sync.dma_start(out=scf[:ntail, 4 * nfull:4 * nfull + 4],
                              in_=sc_full[nfull * 128:N, :])
            nc.sync.dma_start(out=xsb[nfull * 128:N, 64:68],
                              in_=scf[:ntail, 4 * nfull:4 * nfull + 4])

        # ---------- per-window self-loop contribution (PSUM init) ----------
        def open_window(layer, w):
            if layer == 1:
                agg_ps = psA.tile([128, AW], F32, tag="agg")
                den_ps = psA.tile([H, win], F32, tag="den", bufs=1)
                t0 = work.tile([128, 4], F32, tag="t0")
                nc.vector.tensor_tensor(out=t0[:, :], in0=as1_blk[:, 4 * w:4 * w + 4],
                                        in1=ad1f[:, 4 * w:4 * w + 4],
                                        op=mybir.AluOpType.add)
                nc.vector.scalar_tensor_tensor(
                    out=t0[:, :], in0=t0[:, :], scalar=cfg.neg_slope,
                    in1=t0[:, :], op0=mybir.AluOpType.mult, op1=mybir.AluOpType.max)
                pb1s = work.tile([128, 4], BF16, tag="pb1s")
                nc.scalar.activation(pb1s[:, :], t0[:, :],
                                     mybir.ActivationFunctionType.Exp)
                pd = work.tile([128, AW], BF16, tag="pd")
                nc.vector.tensor_tensor(
                    out=pd[:, :].rearrange("p (d h) -> p d h", h=H),
                    in0=I128b[:, :].to_broadcast([128, 128, H]),
                    in1=pb1s[:, :].to_broadcast([128, H, 128]).rearrange(
                        "p h d -> p d h"),
                    op=mybir.AluOpType.mult)
                nc.tensor.matmul(agg_ps[:, :], xlb_all[:, 128 * w:128 * w + 128],
                                 pd[:, :], start=True, stop=False)
                nc.tensor.matmul(den_ps[:, :], pb1s[:, :], I128b[:, 0:win],
                                 start=True, stop=False)
                return agg_ps, den_ps
            else:
                P2 = psA.tile([L2C, win], F32, tag="agg")
                t0 = work.tile([128, 1], F32, tag="t0b")
                nc.vector.tensor_tensor(out=t0[:, :], in0=as2_self[:, w:w + 1],
                                        in1=ad2f[:, w:w + 1], op=mybir.AluOpType.add)
                nc.vector.scalar_tensor_tensor(
                    out=t0[:, :], in0=t0[:, :], scalar=cfg.neg_slope,
                    in1=t0[:, :], op0=mybir.AluOpType.mult, op1=mybir.AluOpType.max)
                pb2s = work.tile([128, 1], BF16, tag="pb2s")
                nc.scalar.activation(pb2s[:, :], t0[:, :],
                                     mybir.ActivationFunctionType.Exp)
                pd2 = work.tile([128, win], BF16, tag="pd2")
                nc.vector.tensor_tensor(
                    out=pd2[:, :], in0=I128b[:, 0:win],
                    in1=pb2s[:, :].to_broadcast([128, win]),
                    op=mybir.AluOpType.mult)
                nc.tensor.matmul(P2[:, :], rowb_all[:, L2C * w:L2C * (w + 1)],
                                 pd2[:, :], start=True, stop=False)
                return P2, None

        # ---------- edge phase (shared schedule for both layers) ----------
        def edge_phase(layer):
            nh = H if layer == 1 else 1
            src_rows = xsb if layer == 1 else h1_full
            esz = 128 if layer == 1 else 64

            # tile -> (window, span, half, global tile idx) flat list
            tiles = []
            for (w, k, f, T) in sched:
                for t in range(T):
                    tiles.append((w, k, f))
            # issue gather calls grouped; consume per window
            call_of_tile = {}
            for ci, (t0c, ntc, f) in enumerate(calls):
                for t in range(t0c, t0c + ntc):
                    call_of_tile[t] = ci

            xg_strips = {}
            st_strips = {}

            def issue_call(ci):
                t0c, ntc, f = calls[ci]
                xg = xgp.tile([128, CALL_CAP * esz], F32, tag=f"xg{layer}")
                base = src_rows[0:N, :] if f == 0 else src_rows[HS:N, :]
                nc.gpsimd.dma_gather(
                    out_ap=xg[:, 0:ntc * esz].rearrange(
                        "p (c e) -> p c e", c=ntc, e=esz),
                    in_ap=base,
                    idxs_ap=idx_sb[:, 8 * t0c:8 * (t0c + ntc)],
                    num_idxs=ntc * 128, num_idxs_reg=ntc * 128, elem_size=esz,
                    queue_num=ci % NQ)
                xg_strips[ci] = xg
                st = stream.tile([128, CALL_CAP * 128], BF16, tag="st")
                nc.sync.dma_start(out=st[:, 0:ntc * 128],
                                  in_=ST_d[:, 128 * t0c:128 * (t0c + ntc)])
                st_strips[ci] = st

            # software-pipelined walk: score chains run one batch ahead
            # of the agg/den matmuls so PE never stalls on DVE/ACT.
            units = []
            gt = 0
            si = 0
            while si < len(sched):
                w = sched[si][0]
                grp = []
                while si < len(sched) and sched[si][0] == w:
                    grp.append(sched[si])
                    si += 1
                wtiles = []
                for (w_, k, f, T) in grp:
                    for t in range(T):
                        wtiles.append((k, gt))
                        gt += 1
                bs = [wtiles[b0:b0 + tb] for b0 in range(0, len(wtiles), tb)]
                wlast = wtiles[-1][1]
                for bi_, b in enumerate(bs):
                    units.append((w, b, bi_ == 0, bi_ == len(bs) - 1, wlast))

            def emit_scores(w, batch):
                nb = len(batch)
                ad_ps = psA.tile([128, tb * nh], F32, tag="ad")
                for bi, (k, t) in enumerate(batch):
                    ci_ = call_of_tile[t]
                    soff = t - calls[ci_][0]
                    st_sb = st_strips[ci_]
                    rhs = (ad1_all[:, 4 * w:4 * w + 4] if layer == 1
                           else ad2_all[:, w:w + 1])
                    nc.tensor.matmul(
                        ad_ps[:, nh * bi:nh * (bi + 1)],
                        st_sb[:, 128 * soff:128 * (soff + 1)], rhs,
                        start=True, stop=True)
                scs = work.tile([128, tb * nh], F32, tag="scs")
                for bi, (k, t) in enumerate(batch):
                    ci = call_of_tile[t]
                    xg = xg_strips[ci]
                    toff = t - calls[ci][0]
                    if layer == 1:
                        a_s_ap = xg[:, esz * toff + 64:esz * toff + 64 + nh]
                    else:
                        a_s_ap = xg[:, esz * toff + 5:esz * toff + 6].bitcast(
                            BF16)[:, 0:1]
                    nc.vector.tensor_tensor(
                        out=scs[:, nh * bi:nh * (bi + 1)], in0=a_s_ap,
                        in1=ad_ps[:, nh * bi:nh * (bi + 1)],
                        op=mybir.AluOpType.add)
                nc.vector.scalar_tensor_tensor(
                    out=scs[:, 0:nb * nh], in0=scs[:, 0:nb * nh],
                    scalar=cfg.neg_slope, in1=scs[:, 0:nb * nh],
                    op0=mybir.AluOpType.mult, op1=mybir.AluOpType.max)
                p_bf = work.tile([128, tb * nh], BF16, tag="pbf")
                nc.scalar.activation(p_bf[:, 0:nb * nh], scs[:, 0:nb * nh],
                                     mybir.ActivationFunctionType.Exp)
                s4g = work.tile([128, tb * nh * ws], BF16, tag="s4g")
                t_first = batch[0][1]
                nc.vector.tensor_tensor(
                    out=s4g[:, 0:nb * nh * ws].rearrange(
                        "p (t s h) -> p t s h", s=ws, h=nh),
                    in0=S_sb[:, ws * t_first:ws * (t_first + nb)].rearrange(
                        "p (t s) -> p t s", s=ws).to_broadcast(
                        [128, nb, ws, nh]),
                    in1=p_bf[:, 0:nb * nh].rearrange(
                        "p (t h) -> p t h", h=nh).to_broadcast(
                        [128, nb, nh, ws]).rearrange("p t h s -> p t s h"),
                    op=mybir.AluOpType.mult)
                return p_bf, s4g

            def emit_agg(w, batch, p_bf, s4g, agg_ps, den_ps):
                for bi, (k, t) in enumerate(batch):
                    ci = call_of_tile[t]
                    xg = xg_strips[ci]
                    toff = t - calls[ci][0]
                    pb_ = p_bf[:, nh * bi:nh * (bi + 1)]
                    s4 = s4g[:, nh * ws * bi:nh * ws * (bi + 1)]
                    Ssl = S_sb[:, ws * t:ws * (t + 1)]
                    if layer == 1:
                        nc.tensor.matmul(
                            agg_ps[:, 4 * ws * k:4 * ws * k + nh * ws],
                            xg[:, esz * toff:esz * toff + 64].bitcast(BF16),
                            s4, start=False, stop=False,
                            skip_group_check=True)
                        nc.tensor.matmul(
                            den_ps[0:nh, ws * k:ws * (k + 1)], pb_, Ssl,
                            start=False, stop=False, skip_group_check=True)
                    else:
                        nc.tensor.matmul(
                            agg_ps[0:L2C, ws * k:ws * (k + 1)],
                            xg[:, esz * toff:esz * toff + 6].bitcast(BF16),
                            s4, start=False, stop=False,
                            skip_group_check=True)

            next_call = 0
            cur_psums = None
            for (w, batch, first, last, wlast) in units:
                if first:
                    while (next_call < len(calls)
                           and calls[next_call][0] <= wlast):
                        issue_call(next_call)
                        next_call += 1
                    cur_psums = open_window(layer, w)
                p_bf, s4g = emit_scores(w, batch)
                emit_agg(w, batch, p_bf, s4g, cur_psums[0], cur_psums[1])
                if last:
                    finish_window(layer, w, cur_psums[0], cur_psums[1])


        # ---------- window epilogues (unchanged from dst-window design) ----
        def finish_window(layer, w, agg_ps, den_ps):
            nr = nrows_of(w)
            if layer == 1:
                nh = H
                nc.tensor.matmul(agg_ps[:, :], onesb[:, :], zerosb[:, 0:AW],
                                 start=False, stop=True)
                nc.tensor.matmul(den_ps[:, :], onesb[:, 0:nh], zerosb[:, 0:win],
                                 start=False, stop=True)
                agg_bf = work.tile([128, AW], BF16, tag="aggbf")
                nc.vector.tensor_copy(agg_bf[:, :], agg_ps[:, :])
                den_sb = work.tile([nh, win], F32, tag="densb")
                nc.vector.tensor_copy(den_sb[:, :], den_ps[:, :])
                ncols = H * cfg.HID
                hp = psB.tile([128, 128], F32, tag="hp")
                nc.tensor.matmul(hp[:, 0:ncols], den_sb[:, :], B1r[:, :],
                                 start=True, stop=False)
                for h in range(H):
                    lhs = agg_bf[:, :].rearrange(
                        "p (s h) -> p s h", h=nh)[:, :, h]
                    nc.tensor.matmul(hp[:, 32 * h:32 * h + 32], lhs,
                                     W1b[:, 32 * h:32 * h + 32],
                                     start=False, stop=False,
                                     skip_group_check=True)
                nc.tensor.matmul(hp[:, 0:ncols], onesb[:, :],
                                 zerosb[:, 0:ncols], start=False, stop=True)
                dT_ps = psB.tile([128, 4], F32, tag="scr")
                nc.tensor.transpose(dT_ps[:win, 0:nh], den_sb[:, :], I128[:nh, :nh])
                rec = work.tile([128, 4], F32, tag="rec")
                nc.vector.tensor_copy(rec[:win, 0:nh], dT_ps[:win, 0:nh])
                nc.vector.reciprocal(rec[:win, 0:nh], rec[:win, 0:nh])
                hn = work.tile([128, 128], F32, tag="hn")
                nc.vector.tensor_tensor(
                    out=hn[:nr, 0:ncols].rearrange("p (h c) -> p h c", h=nh),
                    in0=hp[:nr, 0:ncols].rearrange("p (h c) -> p h c", h=nh),
                    in1=rec[:nr, 0:nh].to_broadcast([nr, nh, cfg.HID]),
                    op=mybir.AluOpType.mult)
                t1 = work.tile([128, 128], F32, tag="t1")
                nc.vector.tensor_scalar_min(t1[:nr, 0:ncols], hn[:nr, 0:ncols], 0.0)
                nc.scalar.activation(t1[:nr, 0:ncols], t1[:nr, 0:ncols],
                                     mybir.ActivationFunctionType.Exp)
                nc.vector.scalar_tensor_tensor(
                    out=t1[:nr, 0:ncols], in0=hn[:nr, 0:ncols], scalar=0.0,
                    in1=t1[:nr, 0:ncols], op0=mybir.AluOpType.max,
                    op1=mybir.AluOpType.add)
                h1bf = work.tile([128, 128], BF16, tag="h1bf")
                nc.vector.tensor_scalar_add(h1bf[:nr, 0:ncols], t1[:nr, 0:ncols],
                                            -1.0)
                hT_ps = psB.tile([128, 128], BF16, tag="scrb")
                nc.tensor.transpose(hT_ps[:, :nr], h1bf[:nr, 0:ncols], I128b[:nr, :nr])
                hT = work.tile([128, 128], BF16, tag="hT")
                nc.vector.tensor_copy(hT[:, :nr], hT_ps[:, :nr])
                a2_ps = psB.tile([128, L2C], F32, tag="scr")
                nc.tensor.matmul(a2_ps[:nr, :], hT[:, :nr], V2W_sb[:, :],
                                 start=True, stop=True)
                rowb = work.tile([128, 16], BF16, tag="rowbw")
                nc.vector.memset(rowb[:, :], 0.0)
                nc.vector.tensor_copy(rowb[:nr, 0:NCLS + 1], a2_ps[:nr, 0:NCLS + 1])
                nc.vector.memset(rowb[:nr, NCLS + 1:NCLS + 2], 1.0)
                nc.vector.tensor_copy(rowb_all[:nr, L2C * w:L2C * (w + 1)],
                                      rowb[:nr, 0:L2C])
                nc.vector.tensor_copy(ad2_all[:nr, w:w + 1],
                                      a2_ps[:nr, NCLS + 1:NCLS + 2])
                nc.vector.tensor_copy(ad2f[:nr, w:w + 1],
                                      a2_ps[:nr, NCLS + 1:NCLS + 2])
                nc.vector.tensor_copy(as2_self[:nr, w:w + 1],
                                      a2_ps[:nr, NCLS:NCLS + 1])
                nc.sync.dma_start(out=h1_sh[w * win:w * win + nr, 0:8],
                                  in_=rowb[:nr, :].bitcast(F32))
            else:
                P2 = agg_ps
                nc.tensor.matmul(P2[:, :], onesb[:, 0:L2C], zerosb[:, 0:win],
                                 start=False, stop=True)
                P2sb = work.tile([L2C, win], F32, tag="p2sb")
                nc.vector.tensor_copy(P2sb[:, :], P2[:, :])
                hpT_ps = psB.tile([128, 16], F32, tag="scr")
                nc.tensor.transpose(hpT_ps[:win, 0:L2C], P2sb[:, :], I128[:L2C, :L2C])
                hpT = work.tile([128, 16], F32, tag="hpT")
                nc.vector.tensor_copy(hpT[:win, 0:L2C], hpT_ps[:win, 0:L2C])
                rec2 = work.tile([128, 1], F32, tag="rec2")
                nc.vector.reciprocal(rec2[:nr, 0:1], hpT[:nr, NCLS + 1:NCLS + 2])
                hn2 = work.tile([128, NCLS], F32, tag="hn2")
                nc.vector.tensor_scalar_mul(hn2[:nr, :], hpT[:nr, 0:NCLS],
                                            rec2[:nr, 0:1])
                nc.vector.tensor_tensor(out=hn2[:nr, :], in0=hn2[:nr, :],
                                        in1=b2rep[:nr, :], op=mybir.AluOpType.add)
                nc.sync.dma_start(out=out_d[w * win:w * win + nr, :],
                                  in_=hn2[:nr, :])

        _ph = os.environ.get("GAT_PHASES", "2")
        if _ph >= "1":
            edge_phase(1)
        if _ph >= "2":
            nc.gpsimd.collective_compute(
                "AllGather", mybir.AluOpType.bypass,
                replica_groups=[list(range(cfg.n_cores))],
                ins=[h1_sh.opt()], outs=[h1_full.opt()])
            edge_phase(2)

    nc.compile()
    return nc


def make_inputs(cfg, x, edge_index, W1, as1, ad1, b1, W2, as2, ad2, b2):
    x = np.asarray(x, np.float32)
    sched, calls, TT, aux = preprocess(cfg, edge_index)
    V1, V2W = fold_params(cfg, np.asarray(W1, np.float32), np.asarray(as1, np.float32),
                          np.asarray(ad1, np.float32), np.asarray(W2, np.float32),
                          np.asarray(as2, np.float32), np.asarray(ad2, np.float32))
    H, HID, NCLS = cfg.HEADS, cfg.HID, cfg.NCLS
    xsb = np.zeros((cfg.N, 128), np.float32)
    xsb[:, :64] = x.astype(ml_dtypes.bfloat16).view(np.float32)
    B1r = np.zeros((H, H * HID), np.float32)
    for h in range(H):
        B1r[h, 32 * h:32 * h + 32] = np.asarray(b1, np.float32)[32 * h:32 * h + 32]
    b2rep = np.tile(np.asarray(b2, np.float32).reshape(1, NCLS), (128, 1))
    shared = dict(
        xsb=xsb, Vsd1=V1.astype(ml_dtypes.bfloat16),
        V2W=V2W.astype(ml_dtypes.bfloat16),
        W1b=np.asarray(W1, np.float32).astype(ml_dtypes.bfloat16),
        B1r=B1r, b2rep=b2rep,
        I128=np.eye(128, dtype=np.float32),
        I128b=np.eye(128, dtype=ml_dtypes.bfloat16),
        onesb=np.ones((128, 128), ml_dtypes.bfloat16),
        Ek4=_ek4(),
        zerosb=np.zeros((128, H * cfg.win), ml_dtypes.bfloat16),
    )
    in_maps = []
    for c in range(cfg.n_cores):
        m = dict(shared)
        m["x_loc"] = x[c * cfg.shard:(c + 1) * cfg.shard]
        m["idx"] = aux[c]["idx"]
        m["S"] = aux[c]["S"]
        m["ST"] = aux[c]["ST"]
        in_maps.append(m)
    return sched, calls, TT, in_maps


LAST_EXEC_NS = [None]


def run_gat(cfg, inputs, nc_cache=[None, None]):
    sched, calls, TT, in_maps = make_inputs(
        cfg, inputs["x"], inputs["edge_index"], inputs["W1"], inputs["att_src1"],
        inputs["att_dst1"], inputs["b1"], inputs["W2"], inputs["att_src2"],
        inputs["att_dst2"], inputs["b2"])
    key = (tuple(sched), tuple(calls))
    if nc_cache[0] != key:
        nc_cache[0] = key
        nc_cache[1] = build_program(cfg, sched, calls, TT)
    nc = nc_cache[1]
    trace = os.environ.get("GAT_TRACE", "0") == "1"
    res = run_bass_kernel_spmd(nc, in_maps, list(range(cfg.n_cores)), trace=trace)
    LAST_EXEC_NS[0] = res.exec_time_ns
    if trace and res.instructions_and_trace:
        print("TRACE PATH:", res.instructions_and_trace[1])
        print("PROFILE JSON:", res.profile_json)
    out = np.concatenate([res.results[c]["out"] for c in range(cfg.n_cores)], axis=0)
    return out.astype(np.float32)


def kernel(**inputs):
    cfg = Cfg(N=50000, E=800000, IN=128, HID=32, HEADS=4, NCLS=10)
    return run_gat(cfg, inputs)
